# revision 9
# baseline (speedup 1.0000x reference)
"""Trainium2 Bass kernel for nn_AffinityLayer (GRU-like recurrent layer).

Math restructure: cat = [h, x_t], W = [Wh | Wx] (fan-in split), so
  cat @ W.T = h @ Wh.T + x_t @ Wx.T
Phase 1 (time-parallel): U = X @ WxT + b for all (b, t) — one big matmul.
Phase 2 (sequential scan over t): a/g = h @ WhT + U[t], gated blend, LayerNorm.

Sharding: data-parallel over batch: 128 batch / 8 cores = 16 per core.
Recurrent matmuls use float32r (full PE speed, near-fp32 precision) with the
per-step h kept TRANSPOSED ([h=128p x 4chunks, b=16]) via PE transposes, so the
stationary operand is a cheap [128,16] tile and the moving operand is the
resident weight [128,512].
"""

import numpy as np

import concourse.bass as bass
import concourse.bacc as bacc
import concourse.tile as tile
from concourse import mybir
from concourse.bass_utils import run_bass_kernel_spmd
from concourse.masks import make_identity

B, N, XLEN, HLEN = 128, 512, 512, 512
NCORES = 8
BS = B // NCORES  # 16 batch per core
H2 = 2 * HLEN     # a|g stacked out dim
KO = HLEN // 128  # 4 k-chunks of 128
EPS = 1e-5
UCH = 4           # U steps per DMA chunk

F32 = mybir.dt.float32
F32R = mybir.dt.float32r
AF = mybir.ActivationFunctionType
OP = mybir.AluOpType

_CACHE = {}
LAST_EXEC_NS = None


def _build():
    nc = bacc.Bacc("TRN2", target_bir_lowering=False, debug=False)
    xt = nc.dram_tensor("xt", [XLEN, BS * N], F32, kind="ExternalInput")
    wht = nc.dram_tensor("wht", [HLEN, H2], F32, kind="ExternalInput")
    wxt = nc.dram_tensor("wxt", [XLEN, H2], F32, kind="ExternalInput")
    bb = nc.dram_tensor("bb", [128, H2], F32, kind="ExternalInput")
    gb = nc.dram_tensor("gb", [BS, HLEN], F32, kind="ExternalInput")
    btb = nc.dram_tensor("btb", [BS, HLEN], F32, kind="ExternalInput")
    y = nc.dram_tensor("y", [BS, N, HLEN], F32, kind="ExternalOutput")
    u_dram = nc.dram_tensor("u_scratch", [N, BS, H2], F32)

    xt_r = xt.rearrange("(ko p) tok -> p ko tok", p=128)

    with tile.TileContext(nc) as tc:
        with tc.tile_pool(name="consts", bufs=1) as consts:
            wht_st = consts.tile([128, KO, H2], F32)
            nc.sync.dma_start(wht_st[:], wht.rearrange("(ko p) n -> p ko n", p=128))
            wxt_st = consts.tile([128, KO, H2], F32)
            nc.sync.dma_start(wxt_st[:], wxt.rearrange("(ko p) n -> p ko n", p=128))
            wht_sb = consts.tile([128, KO, H2], F32R)
            nc.vector.tensor_copy(out=wht_sb[:], in_=wht_st[:])
            wxt_sb = consts.tile([128, KO, H2], F32R)
            nc.vector.tensor_copy(out=wxt_sb[:], in_=wxt_st[:])
            bb_sb = consts.tile([128, H2], F32)
            nc.sync.dma_start(bb_sb[:], bb[:, :])
            gb_sb = consts.tile([BS, HLEN], F32)
            nc.sync.dma_start(gb_sb[:], gb[:, :])
            btb_sb = consts.tile([BS, HLEN], F32)
            nc.sync.dma_start(btb_sb[:], btb[:, :])
            ident = consts.tile([128, 128], F32)
            make_identity(nc, ident[:])
            eps_sb = consts.tile([BS, 1], F32)
            nc.gpsimd.memset(eps_sb[:], EPS)

            # ---------------- Phase 1: U = X @ WxT + b ----------------
            with tc.tile_pool(name="xp", bufs=3) as xpool, \
                 tc.tile_pool(name="up", bufs=3) as upool, \
                 tc.tile_pool(name="ps1", bufs=2, space="PSUM") as psum1:
                XCH = 8  # token tiles per X chunk load
                xch_sb = None
                for mt in range(BS * N // 128):  # 64 token tiles
                    if mt % XCH == 0:
                        xst = xpool.tile([128, KO, XCH * 128], F32, tag="xst")
                        nc.sync.dma_start(
                            xst[:],
                            xt_r[:, :, mt * 128:(mt + XCH) * 128])
                        xch_sb = xpool.tile([128, KO, XCH * 128], F32R,
                                            tag="xt")
                        nc.vector.tensor_copy(out=xch_sb[:], in_=xst[:])
                    moff = (mt % XCH) * 128
                    pa = psum1.tile([128, HLEN], F32, tag="pa")
                    pg = psum1.tile([128, HLEN], F32, tag="pg")
                    for k in range(KO):
                        nc.tensor.matmul(
                            pa[:], lhsT=xch_sb[:, k, moff:moff + 128],
                            rhs=wxt_sb[:, k, 0:HLEN],
                            start=(k == 0), stop=(k == KO - 1))
                    for k in range(KO):
                        nc.tensor.matmul(
                            pg[:], lhsT=xch_sb[:, k, moff:moff + 128],
                            rhs=wxt_sb[:, k, HLEN:H2],
                            start=(k == 0), stop=(k == KO - 1))
                    ut = upool.tile([128, H2], F32, tag="ut")
                    nc.vector.tensor_tensor(ut[:, 0:HLEN], pa[:],
                                            bb_sb[:, 0:HLEN], OP.add)
                    nc.vector.tensor_tensor(ut[:, HLEN:H2], pg[:],
                                            bb_sb[:, HLEN:H2], OP.add)
                    b_i, t0 = divmod(mt * 128, N)
                    nc.sync.dma_start(u_dram[t0:t0 + 128, b_i, :], ut[:])

            # ---------------- Phase 2: recurrence ----------------
            with tc.tile_pool(name="hp", bufs=3) as hpool, \
                 tc.tile_pool(name="ew", bufs=3) as ew, \
                 tc.tile_pool(name="u2", bufs=2) as upool2, \
                 tc.tile_pool(name="st", bufs=4) as stats, \
                 tc.tile_pool(name="psA", bufs=2, space="PSUM") as psA, \
                 tc.tile_pool(name="psT", bufs=2, space="PSUM") as psT:

                hzero = hpool.tile([128, KO * BS], F32, tag="hz")
                nc.gpsimd.memset(hzero[:], 0.0)
                hT = hpool.tile([128, KO * BS], F32R, tag="hT")
                nc.vector.tensor_copy(out=hT[:], in_=hzero[:])
                u_sb = None
                for t in range(N):
                    if t % UCH == 0:
                        u_sb = upool2.tile([BS, UCH, H2], F32, tag="u_sb")
                        nc.sync.dma_start(
                            u_sb[:],
                            u_dram[t:t + UCH].rearrange("t b h -> b t h"))
                    uc = u_sb[:, t % UCH]

                    pa = psA.tile([BS, HLEN], F32, tag="pa")
                    pg = psA.tile([BS, HLEN], F32, tag="pg")
                    for k in range(KO):
                        nc.tensor.matmul(
                            pa[:], lhsT=hT[:, k * BS:(k + 1) * BS],
                            rhs=wht_sb[:, k, 0:HLEN],
                            start=(k == 0), stop=(k == KO - 1))
                    for k in range(KO):
                        nc.tensor.matmul(
                            pg[:], lhsT=hT[:, k * BS:(k + 1) * BS],
                            rhs=wht_sb[:, k, HLEN:H2],
                            start=(k == 0), stop=(k == KO - 1))

                    g = ew.tile([BS, HLEN], F32, tag="g")
                    nc.vector.tensor_tensor(g[:], pg[:], uc[:, HLEN:H2], OP.add)
                    alpha = ew.tile([BS, HLEN], F32, tag="alpha")
                    nc.scalar.activation(alpha[:], g[:], AF.Sigmoid)
                    a = ew.tile([BS, HLEN], F32, tag="a")
                    nc.vector.tensor_tensor(a[:], pa[:], uc[:, 0:HLEN], OP.add)
                    ta = ew.tile([BS, HLEN], F32, tag="ta")
                    nc.scalar.activation(ta[:], a[:], AF.Tanh)
                    d = ew.tile([BS, HLEN], F32, tag="d")
                    nc.vector.tensor_tensor(d[:], ta[:], a[:], OP.subtract)
                    nc.vector.tensor_tensor(d[:], alpha[:], d[:], OP.mult)
                    htl = ew.tile([BS, HLEN], F32, tag="htl")
                    nc.vector.tensor_tensor(htl[:], a[:], d[:], OP.add)

                    bnst = stats.tile([BS, 6], F32, tag="bnst")
                    nc.vector.bn_stats(bnst[:], htl[:])
                    mv = stats.tile([BS, 2], F32, tag="mv")
                    nc.vector.bn_aggr(mv[:], bnst[:])
                    std = stats.tile([BS, 1], F32, tag="std")
                    nc.scalar.activation(std[:], mv[:, 1:2], AF.Sqrt,
                                         bias=eps_sb[:])
                    rstd = stats.tile([BS, 1], F32, tag="rstd")
                    nc.vector.reciprocal(rstd[:], std[:])
                    xc = ew.tile([BS, HLEN], F32, tag="xc")
                    nc.vector.tensor_scalar(xc[:], htl[:], mv[:, 0:1], None,
                                            OP.subtract)
                    yt = ew.tile([BS, HLEN], F32, tag="yt")
                    nc.vector.scalar_tensor_tensor(yt[:], xc[:], rstd[:],
                                                   gb_sb[:], OP.mult, OP.mult)
                    yo = ew.tile([BS, HLEN], F32, tag="yo")
                    nc.vector.tensor_tensor(yo[:], yt[:], btb_sb[:], OP.add)
                    nc.sync.dma_start(y[:, t, :], yo[:])

                    if t + 1 < N:
                        hT = hpool.tile([128, KO * BS], F32R, tag="hT")
                        pt = psT.tile([128, KO * BS], F32, tag="pt")
                        for k in range(KO):
                            nc.tensor.transpose(
                                pt[:, k * BS:(k + 1) * BS],
                                yo[:, k * 128:(k + 1) * 128],
                                ident[:BS, :BS])
                        nc.vector.tensor_copy(out=hT[:], in_=pt[:])
    nc.compile()
    return nc


def kernel(X, W_a, W_g, b_a, b_g, gamma, beta):
    global LAST_EXEC_NS
    X = np.ascontiguousarray(np.asarray(X, np.float32))
    WT = np.concatenate([np.asarray(W_a, np.float32),
                         np.asarray(W_g, np.float32)], axis=0).T  # [1024,1024]
    wht = np.ascontiguousarray(WT[:HLEN])   # fan-in h rows
    wxt = np.ascontiguousarray(WT[HLEN:])   # fan-in x rows
    bcat = np.concatenate([np.asarray(b_a, np.float32),
                           np.asarray(b_g, np.float32)])
    bb = np.tile(bcat[None, :], (128, 1))
    gbv = np.tile(np.asarray(gamma, np.float32)[None, :], (BS, 1))
    btv = np.tile(np.asarray(beta, np.float32)[None, :], (BS, 1))

    if "nc" not in _CACHE:
        _CACHE["nc"] = _build()
    nc = _CACHE["nc"]

    in_maps = []
    for c in range(NCORES):
        Xs = X[c * BS:(c + 1) * BS]  # [16, 512, 512]
        xts = np.ascontiguousarray(Xs.reshape(BS * N, XLEN).T)  # [512, 8192]
        in_maps.append({"xt": xts, "wht": wht, "wxt": wxt, "bb": bb,
                        "gb": gbv, "btb": btv})

    res = run_bass_kernel_spmd(nc, in_maps, core_ids=list(range(NCORES)))
    LAST_EXEC_NS = getattr(res, "exec_time_ns", None)
    outs = res.results
    return np.concatenate([outs[c]["y"] for c in range(NCORES)], axis=0)
